# revision 1
# baseline (speedup 1.0000x reference)
import numpy as np

EYE_SIZE = 32
PAD = 0.3
LEFT_IDX = np.arange(36, 42)
RIGHT_IDX = np.arange(42, 48)
B, C, H, W = 64, 3, 512, 512
S = EYE_SIZE


def _eye_bbox(lm, idx):
    pts = lm[:, idx, :]
    x_min = pts[:, :, 0].min(axis=1)
    x_max = pts[:, :, 0].max(axis=1)
    y_min = pts[:, :, 1].min(axis=1)
    y_max = pts[:, :, 1].max(axis=1)
    w = x_max - x_min
    h = y_max - y_min
    return (x_min - w * PAD, y_min - h * PAD, x_max + w * PAD, y_max + h * PAD)


def _grid(x1, y1, x2, y2):
    # float32 throughout to mirror the reference's jax f32 arithmetic
    bx1 = np.clip(x1, 0.0, W - 1.0).astype(np.float32)
    by1 = np.clip(y1, 0.0, H - 1.0).astype(np.float32)
    bx2 = np.clip(x2, 0.0, W - 1.0).astype(np.float32)
    by2 = np.clip(y2, 0.0, H - 1.0).astype(np.float32)
    degenerate = (bx2 - bx1 < 1.0) | (by2 - by1 < 1.0)
    xn0 = bx1 / (W - 1) * np.float32(2.0) - np.float32(1.0)
    xn1 = bx2 / (W - 1) * np.float32(2.0) - np.float32(1.0)
    yn0 = by1 / (H - 1) * np.float32(2.0) - np.float32(1.0)
    yn1 = by2 / (H - 1) * np.float32(2.0) - np.float32(1.0)
    t = (np.arange(S, dtype=np.float32) / np.float32(S - 1))
    xs = xn0[:, None] + (xn1 - xn0)[:, None] * t  # (B,S)
    ys = yn0[:, None] + (yn1 - yn0)[:, None] * t
    gx = np.broadcast_to(xs[:, None, :], (x1.shape[0], S, S)).copy()
    gy = np.broadcast_to(ys[:, :, None], (x1.shape[0], S, S)).copy()
    gx[degenerate] = 0.0
    gy[degenerate] = 0.0
    px = np.clip((gx + np.float32(1.0)) * np.float32(0.5) * (W - 1), 0.0, W - 1.0)
    py = np.clip((gy + np.float32(1.0)) * np.float32(0.5) * (H - 1), 0.0, H - 1.0)
    return px.astype(np.float32), py.astype(np.float32)


def _patch_absdiff_sum(pred, target, px, py):
    # gather 4 bilinear corners for pred and target, sum |bilinear(pred)-bilinear(target)|
    x0 = np.floor(px)
    y0 = np.floor(py)
    wx = (px - x0).astype(np.float32)[..., None]  # (B,S,S,1)
    wy = (py - y0).astype(np.float32)[..., None]
    x0i = np.clip(x0, 0, W - 1).astype(np.int64)
    x1i = np.clip(x0 + 1, 0, W - 1).astype(np.int64)
    y0i = np.clip(y0, 0, H - 1).astype(np.int64)
    y1i = np.clip(y0 + 1, 0, H - 1).astype(np.int64)
    b = np.arange(pred.shape[0])[:, None, None]
    # advanced indexing -> (B,S,S,C)
    d_a = pred[b, :, y0i, x0i] - target[b, :, y0i, x0i]
    d_b = pred[b, :, y0i, x1i] - target[b, :, y0i, x1i]
    d_c = pred[b, :, y1i, x0i] - target[b, :, y1i, x0i]
    d_d = pred[b, :, y1i, x1i] - target[b, :, y1i, x1i]
    diff = (d_a * (1 - wx) * (1 - wy) + d_b * wx * (1 - wy)
            + d_c * (1 - wx) * wy + d_d * wx * wy)
    return np.abs(diff.astype(np.float32)).sum(dtype=np.float64)


def _host_loss(pred, target, landmarks):
    lx1, ly1, lx2, ly2 = _eye_bbox(landmarks, LEFT_IDX)
    rx1, ry1, rx2, ry2 = _eye_bbox(landmarks, RIGHT_IDX)
    n = pred.shape[0] * C * S * S
    lpx, lpy = _grid(lx1, ly1, lx2, ly2)
    rpx, rpy = _grid(rx1, ry1, rx2, ry2)
    sl = _patch_absdiff_sum(pred, target, lpx, lpy)
    sr = _patch_absdiff_sum(pred, target, rpx, rpy)
    return np.float32((sl / n + sr / n) / 2.0)


def kernel(pred, target, landmarks):
    pred = np.asarray(pred, dtype=np.float32)
    target = np.asarray(target, dtype=np.float32)
    landmarks = np.asarray(landmarks, dtype=np.float32)
    return np.asarray(_host_loss(pred, target, landmarks), dtype=np.float32)



# revision 8
# speedup vs baseline: 3179.9732x; 3179.9732x over previous
"""GazeLoss Trainium2 kernel.

Strategy: the bilinear eye-patch sampling is separable (grid x-coords depend
only on the patch column, y-coords only on the patch row), and each sampled
patch row needs exactly two adjacent image rows (y0, y0+1).  So instead of
streaming the full 402 MB of pred+target, each core dma_gathers only the
4 KB row-pair blocks its patches touch (~12 MB/core), then does the bilinear
interpolation + L1 reduction on-chip:

  - batch-parallel across 8 cores (8 images per core, as 2 quads of 4)
  - dma_gather: 768 x 4KB row-pair blocks per quad per tensor
    (partition = (batch_in_quad, patch_row), group = (eye, channel))
  - ap_gather: pick the 2x32 bilinear x-corner columns per patch row
  - DVE: y-interp with per-partition (a,b) weights, x-interp with wx,
    abs+sum reduction to a (128,4) partial per core
  - host: sum partials, normalize.

Host-side work is only landmark->index/weight math (64x68x2 floats).
"""

import sys
import numpy as np

# ---------------------------------------------------------------- constants
EYE_S = 32
PAD = 0.3
LEFT_IDX = np.arange(36, 42)
RIGHT_IDX = np.arange(42, 48)
B, C, H, W = 64, 3, 512, 512
N_CORES = 8
BPC = B // N_CORES          # 8 batches per core
QUADS = 2                   # 2 quads of 4 batches
NROWS = BPC * C * H         # 12288 rows per core (flattened (b,c,y))
GATHER_IDX = 6 * 128        # 768 row-pair blocks per quad per tensor
XG_IDX = 3 * 2 * 2 * EYE_S  # 384 ap_gather indices (c, yhalf, xcorner, jx)

for _p in ("/opt/trn_rl_repo", "/root/.axon_site/_ro/trn_rl_repo"):
    if _p not in sys.path:
        sys.path.append(_p)


def _install_ntff_hook_shim():
    """Provide antenv.axon_hooks if the container's antenv stub lacks it, so
    run_bass_kernel_spmd(trace=True) can capture NTFF profiles via ctypes."""
    import types, contextlib, ctypes, os

    try:
        from antenv.axon_hooks import get_axon_ntff_profile_hook  # noqa: F401
        return
    except ImportError:
        pass

    _hook_holder = {"hook": None}
    so_path = "/opt/axon/libaxon_pjrt.so"
    if os.path.exists(so_path):
        try:
            lib = ctypes.CDLL(so_path)
            if hasattr(lib, "axon_start_nrt_profile"):
                lib.axon_start_nrt_profile.argtypes = [
                    ctypes.POINTER(ctypes.c_int64), ctypes.c_size_t]
                lib.axon_start_nrt_profile.restype = ctypes.c_int64
                lib.axon_stop_nrt_profile.argtypes = [ctypes.c_char_p]
                lib.axon_stop_nrt_profile.restype = ctypes.c_int64

                @contextlib.contextmanager
                def _hook(output_dir, device_ids):
                    import jax
                    jax.devices()
                    if device_ids:
                        ids = (ctypes.c_int64 * len(device_ids))(*device_ids)
                        rc = lib.axon_start_nrt_profile(ids, len(device_ids))
                    else:
                        rc = lib.axon_start_nrt_profile(None, 0)
                    if rc != 0:
                        raise RuntimeError(f"axon_start_nrt_profile rc={rc}")
                    try:
                        yield
                    finally:
                        n = lib.axon_stop_nrt_profile(str(output_dir).encode())
                        print(f"ntff profile: {n} file(s) -> {output_dir}",
                              file=sys.stderr)

                _hook_holder["hook"] = _hook
        except OSError:
            pass

    mod = types.ModuleType("antenv.axon_hooks")
    mod.get_axon_ntff_profile_hook = lambda: _hook_holder["hook"]
    mod.set_axon_ntff_profile_hook = lambda h: _hook_holder.__setitem__("hook", h)
    sys.modules["antenv.axon_hooks"] = mod


_install_ntff_hook_shim()


# ------------------------------------------------------- host landmark math
def _eye_bbox(lm, idx):
    pts = lm[:, idx, :]
    x_min = pts[:, :, 0].min(axis=1)
    x_max = pts[:, :, 0].max(axis=1)
    y_min = pts[:, :, 1].min(axis=1)
    y_max = pts[:, :, 1].max(axis=1)
    w = x_max - x_min
    h = y_max - y_min
    return (x_min - w * np.float32(PAD), y_min - h * np.float32(PAD),
            x_max + w * np.float32(PAD), y_max + h * np.float32(PAD))


def _grid_1d(x1, y1, x2, y2):
    """Separable sampling grid: xs/ys pixel coords, each (B, S), f32 mirroring
    the reference's jax f32 arithmetic."""
    S = EYE_S
    bx1 = np.clip(x1, 0.0, W - 1.0).astype(np.float32)
    by1 = np.clip(y1, 0.0, H - 1.0).astype(np.float32)
    bx2 = np.clip(x2, 0.0, W - 1.0).astype(np.float32)
    by2 = np.clip(y2, 0.0, H - 1.0).astype(np.float32)
    degenerate = (bx2 - bx1 < 1.0) | (by2 - by1 < 1.0)
    xn0 = bx1 / np.float32(W - 1) * np.float32(2.0) - np.float32(1.0)
    xn1 = bx2 / np.float32(W - 1) * np.float32(2.0) - np.float32(1.0)
    yn0 = by1 / np.float32(H - 1) * np.float32(2.0) - np.float32(1.0)
    yn1 = by2 / np.float32(H - 1) * np.float32(2.0) - np.float32(1.0)
    t = (np.arange(S, dtype=np.float32) / np.float32(S - 1))
    xs = xn0[:, None] + (xn1 - xn0)[:, None] * t
    ys = yn0[:, None] + (yn1 - yn0)[:, None] * t
    xs[degenerate] = 0.0
    ys[degenerate] = 0.0
    px = np.clip((xs + np.float32(1.0)) * np.float32(0.5) * np.float32(W - 1),
                 0.0, W - 1.0).astype(np.float32)
    py = np.clip((ys + np.float32(1.0)) * np.float32(0.5) * np.float32(H - 1),
                 0.0, H - 1.0).astype(np.float32)
    return px, py


def _build_aux(landmarks):
    """Landmarks -> per-(batch,eye) sampling indices/weights."""
    lms = landmarks.astype(np.float32)
    S = EYE_S
    px = np.zeros((B, 2, S), np.float32)
    py = np.zeros((B, 2, S), np.float32)
    for e, idx in enumerate([LEFT_IDX, RIGHT_IDX]):
        bx1, by1, bx2, by2 = _eye_bbox(lms, idx)
        px[:, e], py[:, e] = _grid_1d(bx1, by1, bx2, by2)
    x0f = np.floor(px)
    wx = (px - x0f).astype(np.float32)
    x0i = np.clip(x0f, 0, W - 1).astype(np.int64)
    x1i = np.clip(x0f + 1, 0, W - 1).astype(np.int64)
    y0f = np.floor(py)
    wy = (py - y0f).astype(np.float32)
    y0i = np.clip(y0f, 0, H - 1).astype(np.int64)
    base = np.minimum(y0i, H - 2)            # row-pair block start
    a = np.where(y0i < H - 1, 1.0 - wy, 0.0).astype(np.float32)  # weight row 0
    b = np.where(y0i < H - 1, wy, 1.0).astype(np.float32)        # weight row 1
    return dict(x0i=x0i, x1i=x1i, wx=wx, base=base, a=a, b=b)


def _wrap16(idx_flat, ncols):
    """dma_gather/ap_gather index layout: idx j -> (partition j%16, col j//16),
    one 16-partition pattern."""
    return np.asarray(idx_flat, np.int16).reshape(ncols, 16).T.copy()


def _pack_core_inputs(pred, target, aux, core):
    """Build the per-core in_map for run_bass_kernel_spmd."""
    S = EYE_S
    x0i, x1i, wx = aux["x0i"], aux["x1i"], aux["wx"]
    base, a, b = aux["base"], aux["a"], aux["b"]
    b0 = core * BPC

    gidx = np.zeros((128, 2 * 48), np.int16)
    xidx = np.zeros((128, 4 * 24), np.int16)
    wab = np.zeros((128, 8), np.float32)
    wx3 = np.zeros((128, 4 * 96), np.float32)

    p_bsub = np.arange(128) // S            # partition -> batch-in-quad
    p_iy = np.arange(128) % S               # partition -> patch row
    for q in range(QUADS):
        bg = b0 + q * 4 + p_bsub            # (128,) global batch per partition
        for e in range(2):
            # dma_gather rows: idx i = g*128 + p, g = eye*3 + c
            # value = (b_local*C + c)*H + base
            wab[:, q * 4 + e * 2 + 0] = a[bg, e, p_iy]
            wab[:, q * 4 + e * 2 + 1] = b[bg, e, p_iy]
            wx3[:, (q * 2 + e) * 96:(q * 2 + e + 1) * 96] = np.tile(
                wx[bg, e, :], (1, 3)).reshape(128, 96)
            # ap_gather idx per 16-partition group (b_sub = grp//2)
            for grp in range(8):
                bgg = b0 + q * 4 + grp // 2
                jidx = np.zeros(XG_IDX, np.int64)
                for c in range(C):
                    for yh in range(2):
                        for xc in range(2):
                            xi = (x0i if xc == 0 else x1i)[bgg, e]
                            jj = ((c * 2 + yh) * 2 + xc) * S + np.arange(S)
                            jidx[jj] = c * 1024 + yh * W + xi
                xidx[grp * 16:(grp + 1) * 16, (q * 2 + e) * 24:(q * 2 + e + 1) * 24] = \
                    _wrap16(jidx, 24)
        gflat = np.zeros(GATHER_IDX, np.int64)
        for g in range(6):
            e, c = divmod(g, 3)
            bl = q * 4 + p_bsub             # core-local batch
            gflat[g * 128:(g + 1) * 128] = (bl * C + c) * H + base[bg, e, p_iy]
        gidx[:, q * 48:(q + 1) * 48] = np.tile(_wrap16(gflat, 48), (8, 1))

    shard = slice(core * BPC, (core + 1) * BPC)
    return {
        "pred": np.ascontiguousarray(pred[shard]).reshape(NROWS, W),
        "target": np.ascontiguousarray(target[shard]).reshape(NROWS, W),
        "gidx": gidx,
        "xidx": xidx,
        "wab": wab,
        "wx3": wx3,
    }


# ------------------------------------------------------------ device module
_MODULE_CACHE = {}


def build_module():
    if "nc" in _MODULE_CACHE:
        return _MODULE_CACHE["nc"]
    from contextlib import ExitStack
    import concourse.bass as bass
    import concourse.tile as tile
    from concourse import bacc
    from concourse.mybir import AluOpType, AxisListType, dt

    f32, i16 = dt.float32, dt.int16
    S = EYE_S
    nc = bacc.Bacc("TRN2", target_bir_lowering=False, debug=False,
                   enable_asserts=False, num_devices=1)
    pred_t = nc.dram_tensor("pred", (NROWS, W), f32, kind="ExternalInput")
    targ_t = nc.dram_tensor("target", (NROWS, W), f32, kind="ExternalInput")
    gidx_t = nc.dram_tensor("gidx", (128, 96), i16, kind="ExternalInput")
    xidx_t = nc.dram_tensor("xidx", (128, 96), i16, kind="ExternalInput")
    wab_t = nc.dram_tensor("wab", (128, 8), f32, kind="ExternalInput")
    wx3_t = nc.dram_tensor("wx3", (128, 384), f32, kind="ExternalInput")
    out_t = nc.dram_tensor("out", (128, 4), f32, kind="ExternalOutput")

    with ExitStack() as ctx:
        tc = ctx.enter_context(tile.TileContext(nc))
        aux = ctx.enter_context(tc.tile_pool(name="aux", bufs=1))
        gpool = ctx.enter_context(tc.tile_pool(name="g", bufs=1))
        xpool = ctx.enter_context(tc.tile_pool(name="x", bufs=4))
        spool = ctx.enter_context(tc.tile_pool(name="s", bufs=4))

        gidx_sb = aux.tile([128, 96], i16)
        xidx_sb = aux.tile([128, 96], i16)
        wab_sb = aux.tile([128, 8], f32)
        wx3_sb = aux.tile([128, 384], f32)
        out_sb = aux.tile([128, 4], f32)
        nc.sync.dma_start(gidx_sb[:], gidx_t.ap())
        nc.sync.dma_start(xidx_sb[:], xidx_t.ap())
        nc.sync.dma_start(wab_sb[:], wab_t.ap())
        nc.sync.dma_start(wx3_sb[:], wx3_t.ap())

        # overlapping row-pair window view: window i = rows [i, i+2)
        def win(t):
            return bass.AP(tensor=t, offset=0, ap=[[W, NROWS - 1], [1, 2 * W]])

        # all row-pair gathers first (one gpsimd library switch total:
        # mlp for dma_gather, then ap_gather lib for the x-column picks)
        gtiles = []
        for q in range(QUADS):
            gp = gpool.tile([128, 6, 2 * W], f32, tag=f"gp{q}")
            gt = gpool.tile([128, 6, 2 * W], f32, tag=f"gt{q}")
            qidx = gidx_sb[:, q * 48:(q + 1) * 48]
            nc.gpsimd.dma_gather(gp[:], win(pred_t), qidx, num_idxs=GATHER_IDX,
                                 num_idxs_reg=GATHER_IDX, elem_size=2 * W,
                                 elem_step=W)
            nc.gpsimd.dma_gather(gt[:], win(targ_t), qidx, num_idxs=GATHER_IDX,
                                 num_idxs_reg=GATHER_IDX, elem_size=2 * W,
                                 elem_step=W)
            gtiles.append((gp, gt))

        for q in range(QUADS):
            gp, gt = gtiles[q]
            for e in range(2):
                xp = xpool.tile([128, C, 2, 2, S], f32, tag="xp")
                xt = xpool.tile([128, C, 2, 2, S], f32, tag="xt")
                eidx = xidx_sb[:, (q * 2 + e) * 24:(q * 2 + e + 1) * 24]
                nc.gpsimd.ap_gather(xp[:], gp[:, e * C:(e + 1) * C, :], eidx,
                                    channels=128, num_elems=C * 2 * W, d=1,
                                    num_idxs=XG_IDX)
                nc.gpsimd.ap_gather(xt[:], gt[:, e * C:(e + 1) * C, :], eidx,
                                    channels=128, num_elems=C * 2 * W, d=1,
                                    num_idxs=XG_IDX)
                a_ap = wab_sb[:, q * 4 + e * 2: q * 4 + e * 2 + 1]
                b_ap = wab_sb[:, q * 4 + e * 2 + 1: q * 4 + e * 2 + 2]
                s0 = spool.tile([128, C, 2, S], f32, tag="s0")
                s1 = spool.tile([128, C, 2, S], f32, tag="s1")
                nc.vector.tensor_sub(s0[:], xp[:, :, 0], xt[:, :, 0])
                nc.vector.tensor_sub(s1[:], xp[:, :, 1], xt[:, :, 1])
                t0 = spool.tile([128, C, 2, S], f32, tag="t0")
                nc.vector.tensor_scalar_mul(t0[:], s0[:], a_ap)
                R = spool.tile([128, C, 2, S], f32, tag="R")
                nc.vector.scalar_tensor_tensor(R[:], s1[:], b_ap, t0[:],
                                               op0=AluOpType.mult,
                                               op1=AluOpType.add)
                u = spool.tile([128, C, S], f32, tag="u")
                nc.vector.tensor_sub(u[:], R[:, :, 1], R[:, :, 0])
                v = spool.tile([128, C, S], f32, tag="v")
                wxe = wx3_sb[:, (q * 2 + e) * 96:(q * 2 + e + 1) * 96]
                nc.vector.tensor_mul(v[:], u[:],
                                     wxe.rearrange("p (c j) -> p c j", c=C))
                wv = spool.tile([128, C, S], f32, tag="wv")
                nc.vector.tensor_add(wv[:], v[:], R[:, :, 0])
                col = q * 2 + e
                nc.vector.tensor_reduce(out_sb[:, col:col + 1], wv[:],
                                        axis=AxisListType.XY, op=AluOpType.add,
                                        apply_absolute_value=True)
        nc.sync.dma_start(out_t.ap(), out_sb[:])

    nc.compile()
    _MODULE_CACHE["nc"] = nc
    return nc


def make_in_maps(pred, target, landmarks):
    aux = _build_aux(np.asarray(landmarks, dtype=np.float32))
    pred = np.asarray(pred, dtype=np.float32)
    target = np.asarray(target, dtype=np.float32)
    return [_pack_core_inputs(pred, target, aux, core) for core in range(N_CORES)]


def finalize(results):
    total = np.float64(0.0)
    for r in results:
        total += r["out"].astype(np.float64).sum()
    n = B * C * EYE_S * EYE_S
    return np.float32((total / n) / 2.0)


def kernel(pred, target, landmarks):
    from concourse import bass_utils
    nc = build_module()
    in_maps = make_in_maps(pred, target, landmarks)
    res = bass_utils.run_bass_kernel_spmd(
        nc, in_maps, core_ids=list(range(N_CORES)), trace=False)
    return finalize(res.results)


# revision 23
# speedup vs baseline: 7003.2428x; 2.2023x over previous
"""GazeLoss Trainium2 kernel.

Strategy: the bilinear eye-patch sampling is separable (grid x-coords depend
only on the patch column, y-coords only on the patch row), and each sampled
patch row needs exactly two adjacent image rows (y0, y0+1).  So instead of
streaming the full 402 MB of pred+target, each core dma_gathers only the
4 KB row-pair blocks its patches touch (~12 MB/core), then does the bilinear
interpolation + L1 reduction on-chip:

  - batch-parallel across 8 cores (8 images per core, as 2 quads of 4)
  - dma_gather: 768 x 4KB row-pair blocks per quad per tensor
    (partition = (batch_in_quad, patch_row), group = (eye, channel))
  - ap_gather: pick the 2x32 bilinear x-corner columns per patch row
  - DVE: y-interp with per-partition (a,b) weights, x-interp with wx,
    abs+sum reduction to a (128,4) partial per core
  - host: sum partials, normalize.

Host-side work is only landmark->index/weight math (64x68x2 floats).
"""

import sys
import numpy as np

# ---------------------------------------------------------------- constants
EYE_S = 32
PAD = 0.3
LEFT_IDX = np.arange(36, 42)
RIGHT_IDX = np.arange(42, 48)
B, C, H, W = 64, 3, 512, 512
N_CORES = 8
BPC = B // N_CORES          # 8 batches per core
QUADS = 2                   # 2 quads of 4 batches
# core-local images are tensor+channel-interleaved:
# (b, y, [pred|target], c, x) -> (4096, 3072): one 24KB row-pair descriptor
# fetches both tensors' 3 channels for a bilinear y-pair.
NROWS = BPC * H             # 4096 interleaved rows per core
RWID = C * W                # 1536 floats per channel-interleaved row
PTW = 2 * RWID              # 3072 floats per (pred|target) row
GATHER_IDX = 128            # row-pair blocks per (quad, eye) gather
XG_IDX = 2 * EYE_S          # 64 ap_gather indices (x0 block | x1 block), d=3

for _p in ("/opt/trn_rl_repo", "/root/.axon_site/_ro/trn_rl_repo"):
    if _p not in sys.path:
        sys.path.append(_p)


def _install_ntff_hook_shim():
    """Provide antenv.axon_hooks if the container's antenv stub lacks it, so
    run_bass_kernel_spmd(trace=True) can capture NTFF profiles via ctypes."""
    import types, contextlib, ctypes, os

    try:
        from antenv.axon_hooks import get_axon_ntff_profile_hook  # noqa: F401
        return
    except ImportError:
        pass

    _hook_holder = {"hook": None}
    so_path = "/opt/axon/libaxon_pjrt.so"
    if os.path.exists(so_path):
        try:
            lib = ctypes.CDLL(so_path)
            if hasattr(lib, "axon_start_nrt_profile"):
                lib.axon_start_nrt_profile.argtypes = [
                    ctypes.POINTER(ctypes.c_int64), ctypes.c_size_t]
                lib.axon_start_nrt_profile.restype = ctypes.c_int64
                lib.axon_stop_nrt_profile.argtypes = [ctypes.c_char_p]
                lib.axon_stop_nrt_profile.restype = ctypes.c_int64

                @contextlib.contextmanager
                def _hook(output_dir, device_ids):
                    import jax
                    jax.devices()
                    if device_ids:
                        ids = (ctypes.c_int64 * len(device_ids))(*device_ids)
                        rc = lib.axon_start_nrt_profile(ids, len(device_ids))
                    else:
                        rc = lib.axon_start_nrt_profile(None, 0)
                    if rc != 0:
                        raise RuntimeError(f"axon_start_nrt_profile rc={rc}")
                    try:
                        yield
                    finally:
                        n = lib.axon_stop_nrt_profile(str(output_dir).encode())
                        print(f"ntff profile: {n} file(s) -> {output_dir}",
                              file=sys.stderr)

                _hook_holder["hook"] = _hook
        except OSError:
            pass

    mod = types.ModuleType("antenv.axon_hooks")
    mod.get_axon_ntff_profile_hook = lambda: _hook_holder["hook"]
    mod.set_axon_ntff_profile_hook = lambda h: _hook_holder.__setitem__("hook", h)
    sys.modules["antenv.axon_hooks"] = mod


_install_ntff_hook_shim()


# ------------------------------------------------------- host landmark math
def _eye_bbox(lm, idx):
    pts = lm[:, idx, :]
    x_min = pts[:, :, 0].min(axis=1)
    x_max = pts[:, :, 0].max(axis=1)
    y_min = pts[:, :, 1].min(axis=1)
    y_max = pts[:, :, 1].max(axis=1)
    w = x_max - x_min
    h = y_max - y_min
    return (x_min - w * np.float32(PAD), y_min - h * np.float32(PAD),
            x_max + w * np.float32(PAD), y_max + h * np.float32(PAD))


def _grid_1d(x1, y1, x2, y2):
    """Separable sampling grid: xs/ys pixel coords, each (B, S), f32 mirroring
    the reference's jax f32 arithmetic."""
    S = EYE_S
    bx1 = np.clip(x1, 0.0, W - 1.0).astype(np.float32)
    by1 = np.clip(y1, 0.0, H - 1.0).astype(np.float32)
    bx2 = np.clip(x2, 0.0, W - 1.0).astype(np.float32)
    by2 = np.clip(y2, 0.0, H - 1.0).astype(np.float32)
    degenerate = (bx2 - bx1 < 1.0) | (by2 - by1 < 1.0)
    xn0 = bx1 / np.float32(W - 1) * np.float32(2.0) - np.float32(1.0)
    xn1 = bx2 / np.float32(W - 1) * np.float32(2.0) - np.float32(1.0)
    yn0 = by1 / np.float32(H - 1) * np.float32(2.0) - np.float32(1.0)
    yn1 = by2 / np.float32(H - 1) * np.float32(2.0) - np.float32(1.0)
    t = (np.arange(S, dtype=np.float32) / np.float32(S - 1))
    xs = xn0[:, None] + (xn1 - xn0)[:, None] * t
    ys = yn0[:, None] + (yn1 - yn0)[:, None] * t
    xs[degenerate] = 0.0
    ys[degenerate] = 0.0
    px = np.clip((xs + np.float32(1.0)) * np.float32(0.5) * np.float32(W - 1),
                 0.0, W - 1.0).astype(np.float32)
    py = np.clip((ys + np.float32(1.0)) * np.float32(0.5) * np.float32(H - 1),
                 0.0, H - 1.0).astype(np.float32)
    return px, py


def _build_aux(landmarks):
    """Landmarks -> per-(batch,eye) sampling indices/weights."""
    lms = landmarks.astype(np.float32)
    S = EYE_S
    px = np.zeros((B, 2, S), np.float32)
    py = np.zeros((B, 2, S), np.float32)
    for e, idx in enumerate([LEFT_IDX, RIGHT_IDX]):
        bx1, by1, bx2, by2 = _eye_bbox(lms, idx)
        px[:, e], py[:, e] = _grid_1d(bx1, by1, bx2, by2)
    x0f = np.floor(px)
    wx = (px - x0f).astype(np.float32)
    x0i = np.clip(x0f, 0, W - 1).astype(np.int64)
    x1i = np.clip(x0f + 1, 0, W - 1).astype(np.int64)
    y0f = np.floor(py)
    wy = (py - y0f).astype(np.float32)
    y0i = np.clip(y0f, 0, H - 1).astype(np.int64)
    base = np.minimum(y0i, H - 2)            # row-pair block start
    a = np.where(y0i < H - 1, 1.0 - wy, 0.0).astype(np.float32)  # weight row 0
    b = np.where(y0i < H - 1, wy, 1.0).astype(np.float32)        # weight row 1
    return dict(x0i=x0i, x1i=x1i, wx=wx, base=base, a=a, b=b)


def _wrap16(idx_flat, ncols):
    """dma_gather/ap_gather index layout: idx j -> (partition j%16, col j//16),
    one 16-partition pattern."""
    return np.asarray(idx_flat, np.int16).reshape(ncols, 16).T.copy()


def _pack_core_inputs(pred, target, aux, core):
    """Build the per-core in_map for run_bass_kernel_spmd."""
    S = EYE_S
    x0i, x1i, wx = aux["x0i"], aux["x1i"], aux["wx"]
    base, a, b = aux["base"], aux["a"], aux["b"]
    b0 = core * BPC

    gidx = np.zeros((128, 8 * 8), np.int16)
    xidx = np.zeros((128, 4 * 4), np.int16)
    wab = np.zeros((128, 8), np.float32)
    wx3 = np.zeros((128, 4 * 96), np.float32)

    p_bsub = np.arange(128) // S            # partition -> batch-in-quad
    p_iy = np.arange(128) % S               # partition -> patch row
    for q in range(QUADS):
        bg = b0 + q * 4 + p_bsub            # (128,) global batch per partition
        bl = q * 4 + p_bsub                 # core-local batch
        for e in range(2):
            r = q * 2 + e
            wab[:, q * 4 + e * 2 + 0] = a[bg, e, p_iy]
            wab[:, q * 4 + e * 2 + 1] = b[bg, e, p_iy]
            # wx replicated (jx, c)-major to match the x-major gather output
            wx3[:, r * 96:(r + 1) * 96] = np.repeat(wx[bg, e, :], C, axis=1)
            # dma_gather rows of the (4096, 3072) view, split y0 / y1 so the
            # y-interp can start as soon as the y0 rows land
            rows = bl * H + base[bg, e, p_iy]
            gidx[:, (2 * r) * 8:(2 * r + 1) * 8] = np.tile(
                _wrap16(rows, 8), (8, 1))
            gidx[:, (2 * r + 1) * 8:(2 * r + 2) * 8] = np.tile(
                _wrap16(rows + 1, 8), (8, 1))
            # ap_gather x-corner idx (d=3 units) per 16-partition group
            for grp in range(8):
                bgg = b0 + q * 4 + grp // 2
                jidx = np.concatenate([x0i[bgg, e], x1i[bgg, e]])
                xidx[grp * 16:(grp + 1) * 16, r * 4:(r + 1) * 4] = \
                    _wrap16(jidx, 4)

    # small aux (indices + y-weights) loads first; bulk wx3 second
    aux_u8 = np.zeros((128, 192), np.uint8)
    aux_u8[:, 0:32] = wab.view(np.uint8)
    aux_u8[:, 32:160] = gidx.view(np.uint8)
    aux_u8[:, 160:192] = xidx.view(np.uint8)

    shard = slice(core * BPC, (core + 1) * BPC)
    # (b, y, tensor, c, x) interleaved per-core image rows of 2*3*W floats
    pt = np.empty((BPC, H, 2, C, W), np.float32)
    pt[:, :, 0] = pred[shard].transpose(0, 2, 1, 3)
    pt[:, :, 1] = target[shard].transpose(0, 2, 1, 3)
    return {
        "pt": pt.reshape(NROWS, PTW),
        "aux": aux_u8,
        "wx3": wx3,
    }


# ------------------------------------------------------------ device module
_MODULE_CACHE = {}


def build_module():
    if "nc" in _MODULE_CACHE:
        return _MODULE_CACHE["nc"]
    from contextlib import ExitStack
    import concourse.bass as bass
    import concourse.tile as tile
    from concourse import bacc
    from concourse import mybir as mybir_mod
    from concourse.mybir import AluOpType, AxisListType, dt

    f32, i16, u8 = dt.float32, dt.int16, dt.uint8
    S = EYE_S
    nc = bacc.Bacc("TRN2", target_bir_lowering=False, debug=False,
                   enable_asserts=False, num_devices=1,
                   enable_partition_id=False)
    pt_t = nc.dram_tensor("pt", (NROWS, PTW), f32, kind="ExternalInput")
    aux_t = nc.dram_tensor("aux", (128, 192), u8, kind="ExternalInput")
    wx3_t = nc.dram_tensor("wx3", (128, 384), f32, kind="ExternalInput")
    out_t = nc.dram_tensor("out", (128, 4), f32, kind="ExternalOutput")

    with ExitStack() as ctx:
        tc = ctx.enter_context(tile.TileContext(nc))
        aux = ctx.enter_context(tc.tile_pool(name="aux", bufs=1))
        gpool = ctx.enter_context(tc.tile_pool(name="g", bufs=1))
        xpool = ctx.enter_context(tc.tile_pool(name="x", bufs=4))
        spool = ctx.enter_context(tc.tile_pool(name="s", bufs=3))

        aux_sb = aux.tile([128, 192], u8)
        wx3_sb = aux.tile([128, 384], f32)
        out_sb = aux.tile([128, 4], f32)
        nc.sync.dma_start(aux_sb[:], aux_t.ap())
        nc.sync.dma_start(wx3_sb[:], wx3_t.ap())
        wab_sb = aux_sb[:, 0:32].bitcast(f32)      # (128, 8)
        gidx_sb = aux_sb[:, 32:160].bitcast(i16)   # (128, 64)
        xidx_sb = aux_sb[:, 160:192].bitcast(i16)  # (128, 16)

        # all row gathers first (one gpsimd library switch total:
        # mlp for dma_gather, then the ap_gather lib for the x-column picks)
        gtiles = []
        for q in range(QUADS):
            for e in range(2):
                r = q * 2 + e
                ga = gpool.tile([128, 1, PTW], f32, tag=f"ga{r}")
                gb = gpool.tile([128, 1, PTW], f32, tag=f"gb{r}")
                nc.gpsimd.dma_gather(ga[:], pt_t.ap(),
                                     gidx_sb[:, (2 * r) * 8:(2 * r + 1) * 8],
                                     num_idxs=GATHER_IDX,
                                     num_idxs_reg=GATHER_IDX,
                                     elem_size=PTW, elem_step=PTW)
                nc.gpsimd.dma_gather(gb[:], pt_t.ap(),
                                     gidx_sb[:, (2 * r + 1) * 8:(2 * r + 2) * 8],
                                     num_idxs=GATHER_IDX,
                                     num_idxs_reg=GATHER_IDX,
                                     elem_size=PTW, elem_step=PTW)
                gtiles.append((ga, gb))

        for q in range(QUADS):
            for e in range(2):
                r = q * 2 + e
                ga, gb = gtiles[r]
                a_ap = wab_sb[:, q * 4 + e * 2: q * 4 + e * 2 + 1]
                b_ap = wab_sb[:, q * 4 + e * 2 + 1: q * 4 + e * 2 + 2]
                # y-interp on full interleaved rows: R = a*D0 + b*D1
                # row layout: [pred | targ] x (c, x) 1536
                s0 = spool.tile([128, RWID], f32, tag="s0")
                s1 = spool.tile([128, RWID], f32, tag="s1")
                nc.vector.tensor_sub(s0[:], ga[:, 0, 0:RWID],
                                     ga[:, 0, RWID:2 * RWID])
                nc.vector.tensor_sub(s1[:], gb[:, 0, 0:RWID],
                                     gb[:, 0, RWID:2 * RWID])
                t0 = spool.tile([128, RWID], f32, tag="t0")
                nc.scalar.activation(t0[:], s0[:],
                                     mybir_mod.ActivationFunctionType.Copy,
                                     scale=a_ap)
                # R stored x-major (x, c) so one d=3 gather idx fetches all
                # 3 channels of an x-corner
                Rt = spool.tile([128, W, C], f32, tag="Rt")
                Rv = Rt[:].rearrange("p x c -> p c x")
                nc.vector.scalar_tensor_tensor(
                    Rv, s1[:].rearrange("p (c x) -> p c x", c=C), b_ap,
                    t0[:].rearrange("p (c x) -> p c x", c=C),
                    op0=AluOpType.mult, op1=AluOpType.add)
                # x-corner pick: idx block [x0 | x1], d=3 channels per idx
                xg = xpool.tile([128, 2, S, C], f32, tag="xg")
                eidx = xidx_sb[:, r * 4:(r + 1) * 4]
                nc.gpsimd.ap_gather(xg[:], Rt[:], eidx, channels=128,
                                    num_elems=W, d=C, num_idxs=XG_IDX)
                # x-interp + |.| sum
                u = spool.tile([128, S, C], f32, tag="u")
                nc.vector.tensor_sub(u[:], xg[:, 1], xg[:, 0])
                v = spool.tile([128, S, C], f32, tag="v")
                wxe = wx3_sb[:, r * 96:(r + 1) * 96]
                nc.vector.tensor_mul(v[:], u[:],
                                     wxe.rearrange("p (j c) -> p j c", c=C))
                wv = spool.tile([128, S, C], f32, tag="wv")
                nc.vector.tensor_add(wv[:], v[:], xg[:, 0])
                awv = spool.tile([128, S, C], f32, tag="awv")
                nc.scalar.activation(awv[:], wv[:],
                                     mybir_mod.ActivationFunctionType.Abs,
                                     accum_out=out_sb[:, r:r + 1])
        nc.sync.dma_start(out_t.ap(), out_sb[:])

    nc.compile()
    _MODULE_CACHE["nc"] = nc
    return nc


def make_in_maps(pred, target, landmarks):
    aux = _build_aux(np.asarray(landmarks, dtype=np.float32))
    pred = np.asarray(pred, dtype=np.float32)
    target = np.asarray(target, dtype=np.float32)
    return [_pack_core_inputs(pred, target, aux, core) for core in range(N_CORES)]


def finalize(results):
    total = np.float64(0.0)
    for r in results:
        total += r["out"].astype(np.float64).sum()
    n = B * C * EYE_S * EYE_S
    return np.float32((total / n) / 2.0)


def kernel(pred, target, landmarks):
    from concourse import bass_utils
    nc = build_module()
    in_maps = make_in_maps(pred, target, landmarks)
    res = bass_utils.run_bass_kernel_spmd(
        nc, in_maps, core_ids=list(range(N_CORES)), trace=False)
    return finalize(res.results)
